# revision 1
# baseline (speedup 1.0000x reference)
"""Trainium2 Bass kernel for sparse multi-headed attention.

Semantics (verified against the reference):
  q = x_q @ Wq.T + bq (per head, dk=32), same for k, v
  for each row s: attend to keys {s-c : c in (5,3,1,0), c <= s}
    score_c[s] = q[s].k[s-c] / sqrt(4)
    p = softmax over valid offsets
    attn[s] = sum_c p_c[s] * v[s-c]
  y = attn @ Wo.T + bo

Sharding: data-parallel over d_stock (8 stocks -> 8 cores). Each core
processes 4 (stock,batch) pairs = 2048 rows. Weights replicated.

Device layout: feature-major ("transposed") activations [256 feats, 2048
rows]; the host pre-transposes inputs during the shard step so no on-device
transposes are needed. Scores/softmax live in a pair-block layout
[128 partitions = 4 pair-blocks x (8 heads + 24 unused), 4 offsets x 512].
All matmul inputs are tf32 (float32r, pre-rounded on host); accumulation
and softmax stay fp32.
"""

import numpy as np

from concourse import bacc, bass, mybir, tile
from concourse.bass_utils import run_bass_kernel_spmd

DS, NB, S, DM, H, DK = 8, 4, 512, 256, 8, 32
CONS = (5, 3, 1, 0)
NCORES = 8
NPAIR = NB  # pairs per core (1 stock x 4 batches)
ROWS = NPAIR * S  # 2048
P = 128
PADC = 8  # zero pad columns in front of k/v for shifted reads
NEG = -1e9
SCALE = 0.5  # 1/sqrt(n_att)

f32 = mybir.dt.float32
f32r = mybir.dt.float32r
bf16 = mybir.dt.bfloat16
Act = mybir.ActivationFunctionType


def _emit(ctx, tc, nc, d, y_dram):
    dma_engs = [nc.sync, nc.scalar]
    dma_i = [0]

    def dma(out, in_):
        eng = dma_engs[dma_i[0] % 2]
        dma_i[0] += 1
        eng.dma_start(out=out, in_=in_)

    main = ctx.enter_context(tc.tile_pool(name="main", bufs=1))
    prodp = ctx.enter_context(tc.tile_pool(name="prodp", bufs=8))
    utmpp = ctx.enter_context(tc.tile_pool(name="utmpp", bufs=12))
    smx = ctx.enter_context(tc.tile_pool(name="smx", bufs=4))
    psum_pj = ctx.enter_context(tc.tile_pool(name="pj", bufs=4, space="PSUM"))
    sc_ctx = tc.tile_pool(name="sc", bufs=1, space="PSUM")
    psum_sc = sc_ctx.__enter__()

    # ---------------- PE warmup (HAM un-throttle) while DMAs run ----
    wtile = main.tile([P, 512], bf16, name="wtile")
    nc.vector.memset(wtile[:], 0.0)
    for i in range(20):
        wps = psum_pj.tile([P, 512], f32, name="wps", tag="pjt")
        nc.tensor.matmul(
            wps[:], lhsT=wtile[:, 0:P], rhs=wtile[:], start=True, stop=True)

    # ---------------- loads (q/k first; v deferred) ----------------
    xs = {}
    for name in ("xq", "xk", "xv"):
        for ch in range(2):
            xs[name, ch] = main.tile([P, ROWS], f32r, name=f"{name}{ch}")
    ws = {}
    for name in ("wq", "wk", "wv", "wo"):
        for ch in range(2):
            t = main.tile([P, DM], f32r, name=f"{name}{ch}")
            ws[name, ch] = t
    # ring A carries the q-side, ring B the k-side; slices sized so the
    # first projection matmuls can start ~3us in
    for ch in range(2):
        nc.sync.dma_start(out=ws["wq", ch][:],
                          in_=d["wq"][ch * P:(ch + 1) * P, :])
        nc.scalar.dma_start(out=ws["wk", ch][:],
                            in_=d["wk"][ch * P:(ch + 1) * P, :])
    selkm = []
    bqkv = []
    for ch in range(2):
        t = main.tile([P, 224], f32r, name=f"selkm{ch}")
        nc.sync.dma_start(out=t[:], in_=d["selkm"][ch])
        selkm.append(t)
        t = main.tile([P, 3], f32, name=f"bqkv{ch}")
        nc.scalar.dma_start(out=t[:], in_=d["bqkv"][ch * P:(ch + 1) * P, :])
        bqkv.append(t)
    for lo, hi in ((0, 512), (512, 1024), (1024, 2048)):
        for ch in range(2):
            nc.sync.dma_start(out=xs["xq", ch][:, lo:hi],
                              in_=d["xq"][ch * P:(ch + 1) * P, lo:hi])
            nc.scalar.dma_start(out=xs["xk", ch][:, lo:hi],
                                in_=d["xk"][ch * P:(ch + 1) * P, lo:hi])

    # ---------------- q/k projections (PE, fp32r) ----------------
    # per-pair tiles so downstream work starts as soon as a pair is done;
    # k/v tiles carry an 8-col head holding the previous pair's tail
    qkv = {}
    for name in ("xq", "xk", "xv"):
        for ch in range(2):
            for p in range(NPAIR):
                if name == "xq":
                    qkv[name, ch, p] = main.tile(
                        [P, 512], f32, name=f"pq{ch}_{p}")
                else:
                    t = main.tile(
                        [P, PADC + 512], f32, name=f"p{name[1]}{ch}_{p}")
                    if p == 0:
                        nc.vector.memset(t[:, 0:PADC], 0.0)
                    qkv[name, ch, p] = t

    def project(name, wname, bcol, n, ch):
        ps = psum_pj.tile([P, 512], f32, name="pjt", tag="pjt")
        for kch in range(2):
            nc.tensor.matmul(
                ps[:],
                lhsT=ws[wname, kch][:, ch * P:(ch + 1) * P],
                rhs=xs[name, kch][:, n * 512:(n + 1) * 512],
                start=(kch == 0), stop=(kch == 1))
        bias_ap = bqkv[ch][:, bcol:bcol + 1]
        if name == "xq":
            nc.scalar.activation(
                qkv[name, ch, n][:], ps[:], Act.Identity, bias=bias_ap)
        else:
            nc.scalar.activation(
                qkv[name, ch, n][:, PADC:PADC + 512], ps[:],
                Act.Identity, bias=bias_ap)
            if n < NPAIR - 1:
                nc.scalar.activation(
                    qkv[name, ch, n + 1][:, 0:PADC], ps[:, 512 - PADC:512],
                    Act.Identity, bias=bias_ap)

    for n in range(4):
        for ch in range(2):
            project("xq", "wq", 0, n, ch)
            project("xk", "wk", 1, n, ch)

    # deferred loads: v inputs, selectors for later phases
    for ch in range(2):
        dma(ws["wv", ch][:], d["wv"][ch * P:(ch + 1) * P, :])
        dma(ws["wo", ch][:], d["wo"][ch * P:(ch + 1) * P, :])
    for ch in range(2):
        dma(xs["xv", ch][:], d["xv"][ch * P:(ch + 1) * P, :])
    selmk = []
    for p in range(NPAIR):
        row = []
        for ch in range(2):
            t = main.tile([P, P], f32r, name=f"selmk{p}{ch}")
            dma(t[:], d["selmk"][p, ch])
            row.append(t)
        selmk.append(row)
    ones1 = main.tile([1, P], f32r, name="ones1")
    dma(ones1[:], d["ones1"])
    bo_r = main.tile([1, DM], f32r, name="bo_r")
    dma(bo_r[:], d["bo"])

    # ---------------- scores ----------------
    # sc[32*pair + h, ci*512 + s] = q_h[s] . k_h[s-c] * 0.5
    sc = psum_sc.tile([P, 4 * 512], f32, name="scores")
    # p-major: pair 0's products start as soon as pair 0 is projected.
    # Offsets are processed in stride-uniform pairs (5,3) and (1,0) so one
    # DVE op produces both shifted products ([128, 2, 512], broadcast q).
    for p in range(NPAIR):
        for ch in range(2):
            q_b = qkv["xq", ch, p][:].rearrange(
                "a (o s) -> a o s", o=1).broadcast_to([P, 2, 512])
            k_t = qkv["xk", ch, p]
            for ci0, step in ((0, 2), (2, 1)):
                c_hi = CONS[ci0]
                pr = prodp.tile([P, 2, 512], f32r, name="prod", tag="prod")
                k_ap = k_t[:, PADC - c_hi: PADC - c_hi + step + 512]
                k_v = bass.AP(
                    tensor=k_ap.tensor, offset=k_ap.offset,
                    ap=[k_ap.ap[0], [step, 2], [1, 512]])
                nc.vector.tensor_mul(pr[:], q_b, k_v)
                for j in range(2):
                    ci = ci0 + j
                    nc.tensor.matmul(
                        sc[:, ci * 512:(ci + 1) * 512],
                        lhsT=selkm[ch][:, 96 - 32 * p: 224 - 32 * p],
                        rhs=pr[:, j, :],
                        start=(p == 0 and ch == 0),
                        stop=(p == 3 and ch == 1))
    # mask: scores for s_loc < c -> -1e9 (covers every pair block at once)
    for ci, c in enumerate(CONS):
        if c:
            nc.vector.memset(sc[:, ci * 512: ci * 512 + c], NEG)

    # ---------------- v projection (keeps PE busy during softmax) ----
    for n in range(4):
        for ch in range(2):
            project("xv", "wv", 2, n, ch)

    # ---------------- softmax over the 4 offsets (no max-sub: scores
    # are O(15) and masked lanes exp to 0) ----------------
    p_sb = main.tile([P, 4 * 512], f32r, name="p_sb")
    for ci in range(4):
        nc.scalar.activation(
            p_sb[:, ci * 512:(ci + 1) * 512], sc[:, ci * 512:(ci + 1) * 512],
            Act.Exp)
    sc_ctx.__exit__(None, None, None)
    psum_y = ctx.enter_context(tc.tile_pool(name="yp", bufs=3, space="PSUM"))
    d1 = smx.tile([P, 512], f32, name="d1", tag="smx")
    nc.vector.tensor_add(d1[:], p_sb[:, 0:512], p_sb[:, 512:1024])
    d2 = smx.tile([P, 512], f32, name="d2", tag="smx")
    nc.vector.tensor_add(d2[:], p_sb[:, 1024:1536], p_sb[:, 1536:2048])
    den = smx.tile([P, 512], f32, name="den", tag="smx")
    nc.vector.tensor_add(den[:], d1[:], d2[:])
    rcp = smx.tile([P, 512], f32, name="rcp", tag="smx")
    nc.vector.reciprocal_approx_fast(rcp[:], den[:])
    for ci in range(4):
        nc.vector.tensor_mul(
            p_sb[:, ci * 512:(ci + 1) * 512], p_sb[:, ci * 512:(ci + 1) * 512],
            rcp[:])

    # ---------------- attention + output projection, per pair --------
    y_view = y_dram.rearrange("(n p) d -> p n d", p=P)
    ybig = main.tile([P, 16 * DM], f32, name="ybig")

    def emit_y(p, usum):
        for tl in range(4):  # row-tiles within this pair
            t = 4 * p + tl
            yp = psum_y.tile([P, DM], f32, name="ypt", tag="ypt")
            for ch in range(2):
                nc.tensor.matmul(
                    yp[:],
                    lhsT=usum[ch][:, tl * P:(tl + 1) * P],
                    rhs=ws["wo", ch][:],
                    start=(ch == 0), stop=False)
            nc.tensor.matmul(
                yp[:], lhsT=ones1[:], rhs=bo_r[:], start=False, stop=True)
            nc.scalar.copy(ybig[:, t * DM:(t + 1) * DM], yp[:])
            if tl % 2 == 1:
                t0 = 4 * p + tl - 1
                nc.sync.dma_start(
                    out=y_view[:, t0:t0 + 2, :],
                    in_=ybig[:, t0 * DM:(t0 + 2) * DM].rearrange(
                        "p (n d) -> p n d", n=2))

    prev = None
    for p in range(NPAIR):
        usum = {}
        for ch in range(2):
            uts = []
            for ci, c in enumerate(CONS):
                bc = psum_pj.tile([P, 512], f32, name="bc", tag="pjt")
                nc.tensor.matmul(
                    bc[:],
                    lhsT=selmk[p][ch][:],
                    rhs=p_sb[:, ci * 512:(ci + 1) * 512],
                    start=True, stop=True)
                ut = utmpp.tile([P, 512], f32r, name="ut", tag="ut")
                nc.vector.tensor_mul(
                    ut[:], bc[:],
                    qkv["xv", ch, p][:, PADC - c: PADC + 512 - c])
                uts.append(ut)
            s1 = utmpp.tile([P, 512], f32, name="s1", tag="ut")
            nc.vector.tensor_add(s1[:], uts[0][:], uts[1][:])
            s2 = utmpp.tile([P, 512], f32, name="s2", tag="ut")
            nc.vector.tensor_add(s2[:], uts[2][:], uts[3][:])
            us = utmpp.tile([P, 512], f32r, name="us", tag="ut")
            nc.vector.tensor_add(us[:], s1[:], s2[:])
            usum[ch] = us
        if prev is not None:
            emit_y(prev[0], prev[1])
        prev = (p, usum)
    emit_y(prev[0], prev[1])


def build_nc():
    from contextlib import ExitStack
    nc = bacc.Bacc(trn_type="TRN2", target_bir_lowering=False, debug=False)
    d = {}
    for name in ("xq", "xk", "xv"):
        d[name] = nc.dram_tensor(name, [DM, ROWS], f32r, kind="ExternalInput").ap()
    for name in ("wq", "wk", "wv", "wo"):
        d[name] = nc.dram_tensor(name, [DM, DM], f32r, kind="ExternalInput").ap()
    d["bqkv"] = nc.dram_tensor("bqkv", [DM, 3], f32, kind="ExternalInput").ap()
    d["bo"] = nc.dram_tensor("bo", [1, DM], f32r, kind="ExternalInput").ap()
    d["ones1"] = nc.dram_tensor("ones1", [1, P], f32r, kind="ExternalInput").ap()
    d["selkm"] = nc.dram_tensor("selkm", [2, P, 224], f32r, kind="ExternalInput").ap()
    d["selmk"] = nc.dram_tensor("selmk", [NPAIR, 2, P, P], f32r, kind="ExternalInput").ap()
    y = nc.dram_tensor("y", [ROWS, DM], f32, kind="ExternalOutput").ap()
    with tile.TileContext(nc) as tc:
        with ExitStack() as ctx:
            _emit(ctx, tc, nc, d, y)
    nc.compile()
    return nc


def _round_tf32(a):
    """Round-to-nearest fp32 -> tf32 (10-bit mantissa)."""
    b = np.ascontiguousarray(a, dtype=np.float32).view(np.uint32)
    b = (b + np.uint32(0x1000)) & np.uint32(0xFFFFE000)
    return b.view(np.float32)


def make_shared_inputs(Wq, bq, Wk, bk, Wv, bv, Wo, bo):
    shared = {}
    shared["wq"] = _round_tf32(np.asarray(Wq, np.float32).T)
    shared["wk"] = _round_tf32(np.asarray(Wk, np.float32).T)
    shared["wv"] = _round_tf32(np.asarray(Wv, np.float32).T)
    shared["wo"] = _round_tf32(np.asarray(Wo, np.float32).T)
    shared["bqkv"] = np.ascontiguousarray(
        np.stack([bq, bk, bv], axis=1), dtype=np.float32)
    shared["bo"] = _round_tf32(np.asarray(bo, np.float32).reshape(1, DM))
    shared["ones1"] = np.ones((1, P), np.float32)
    # selkm[ch, d, 96+h] = 0.5 iff h == global head of feature ch*128+d.
    # The score matmul for pair p uses lhsT = selkm[ch][:, 96-32p : 224-32p],
    # whose column j = 32p+h lands the head-h sum on psum partition 32p+h.
    selkm = np.zeros((2, P, 224), np.float32)
    for ch in range(2):
        for dd in range(P):
            selkm[ch, dd, 96 + ch * 4 + dd // 32] = SCALE
    shared["selkm"] = selkm
    # selmk[p, ch, 32p+j, d] = 1 iff global head of feature ch*128+d == j
    selmk = np.zeros((NPAIR, 2, P, P), np.float32)
    for p in range(NPAIR):
        for ch in range(2):
            for dd in range(P):
                selmk[p, ch, 32 * p + ch * 4 + dd // 32, dd] = 1.0
    shared["selmk"] = selmk
    return shared


def make_core_inputs(query, key_in, value, core):
    # core i handles stock i: [4, 512, 256] -> feature-major [256, 2048]
    out = {}
    for name, x in (("xq", query), ("xk", key_in), ("xv", value)):
        xi = np.asarray(x[core], dtype=np.float32).reshape(ROWS, DM)
        out[name] = _round_tf32(xi.T)
    return out


def kernel(query, key_in, value, Wq, bq, Wk, bk, Wv, bv, Wo, bo):
    nc = build_nc()
    shared = make_shared_inputs(Wq, bq, Wk, bk, Wv, bv, Wo, bo)
    in_maps = []
    for core in range(NCORES):
        m = dict(shared)
        m.update(make_core_inputs(query, key_in, value, core))
        in_maps.append(m)
    res = run_bass_kernel_spmd(nc, in_maps, list(range(NCORES))).results
    y = np.stack([res[i]["y"].reshape(NB, S, DM) for i in range(NCORES)])
    return y.astype(np.float32)

